# revision 1
# baseline (speedup 1.0000x reference)
"""Trainium2 Bass kernel for nn_AttentionLearnFusionDirectModule.

Takes FULL inputs, returns FULL output. Shards nseq=16 across 8 NeuronCores
(2 seqs/core, pure data parallel), runs one SPMD Bass program via
bass_utils.run_bass_kernel_spmd, gathers the output on host.

Per-core pipeline (per sequence):
  1. norms: squares (ACT+DVE), PE ones-colsum, DVE quake-rsqrt
  2. cosine sim: f32r matmuls (c,k)x(c,q) per 128-wide k-tile; train-side
     norms folded into the ACT Exp per-partition scale; test-side norms
     pre-multiplied into the moving operand
  3. softmax-free attention: E = exp(temp*cos); num/den via one 16-column
     block-label matmul accumulated over all k-tiles; pmt = num * recip(den)
  4. bilinear 24->96 upsample as two PE matmuls with the resize matrix
  5. mean/std over memories, tanh, triangular soft-binning via a PE
     bin-center matmul + DVE abs_max, then bf16 1x1 convs (BN folded)
"""
import os
import sys

sys.path.insert(0, '/opt/trn_rl_repo')

import numpy as np
from contextlib import ExitStack

import concourse.bass as bass
import concourse.tile as tile
from concourse import mybir, bacc
from concourse.bass_utils import run_bass_kernel_spmd

f32 = mybir.dt.float32
f32r = mybir.dt.float32r
bf16 = mybir.dt.bfloat16
i32 = mybir.dt.int32
AF = mybir.ActivationFunctionType
ALU = mybir.AluOpType

M = 8
NSEQ = 16
C = 256
WF = HF = 24
WL = HL = 96
L = WF * HF            # 576
KTOT = M * L           # 4608
NCORES = 8
SL = NSEQ // NCORES    # 2
BN_EPS = 1e-5
NKT = KTOT // 128      # 36
PIX = WL * HL          # 9216
PIXH = PIX // 2        # 4608
MAGIC = 0x5F3759DF
DEBUG = bool(int(os.environ.get("KERNEL_DEBUG", "0")))
REPEAT = int(os.environ.get("KERNEL_REPEAT", "1"))


def resize_matrix(n_in, n_out):
    """Row-normalized triangle-kernel resize matrix matching jax.image.resize
    bilinear (antialias on downscale, half-pixel centers)."""
    scale = n_out / n_in
    kscale = min(scale, 1.0)
    R = np.zeros((n_out, n_in), np.float64)
    for i in range(n_out):
        c = (i + 0.5) / scale - 0.5
        w = np.maximum(0.0, 1.0 - np.abs(np.arange(n_in) - c) * kscale)
        R[i] = w / w.sum()
    return R.astype(np.float32)


def _rsqrt(nc, pool, x_ap, P, F, tag, n_iter=2):
    """1/sqrt(x) on DVE only (quake init + Newton). x: fp32 SBUF AP."""
    y = pool.tile([P, F], f32, tag=f"rsq_y{tag}")
    t1 = pool.tile([P, F], f32, tag=f"rsq_t{tag}")
    yi = y[:].bitcast(i32)
    nc.vector.tensor_scalar(yi, x_ap.bitcast(i32), 1, None,
                            op0=ALU.logical_shift_right)
    nc.vector.tensor_scalar(yi, yi, 0xFFFFFFFF, None, op0=ALU.bitwise_xor)
    nc.vector.tensor_scalar(yi, yi, MAGIC + 1, None, op0=ALU.add)
    for _ in range(n_iter):
        nc.vector.tensor_tensor(t1[:], y[:], y[:], op=ALU.mult)
        nc.vector.tensor_tensor(t1[:], t1[:], x_ap, op=ALU.mult)
        nc.vector.tensor_scalar(t1[:], t1[:], -0.5, 1.5, op0=ALU.mult, op1=ALU.add)
        nc.vector.tensor_tensor(y[:], y[:], t1[:], op=ALU.mult)
    return y


def build_program():
    nc = bacc.Bacc("TRN2", target_bir_lowering=False, debug=False,
                   num_devices=NCORES)

    trf = nc.dram_tensor("trf", [SL, 2, 128, KTOT], f32r, kind="ExternalInput").ap()
    tef = nc.dram_tensor("tef", [SL, 2, 128, L], f32, kind="ExternalInput").ap()
    lmat = nc.dram_tensor("lmat", [SL, 128, NKT * 40], f32r, kind="ExternalInput").ap()
    tsc = nc.dram_tensor("tsc", [SL, 96, 96], f32, kind="ExternalInput").ap()
    w1 = nc.dram_tensor("w1", [128, 128], bf16, kind="ExternalInput").ap()
    w2 = nc.dram_tensor("w2", [128, 2], bf16, kind="ExternalInput").ap()
    b1 = nc.dram_tensor("b1", [128, 1], f32, kind="ExternalInput").ap()
    binlhs = nc.dram_tensor("binlhs", [8, 128], f32r, kind="ExternalInput").ap()
    utm = nc.dram_tensor("utm", [24, 96], f32r, kind="ExternalInput").ap()
    ones128 = nc.dram_tensor("ones128", [128, 1], f32r, kind="ExternalInput").ap()
    onesrow = nc.dram_tensor("onesrow", [1, 128], f32r, kind="ExternalInput").ap()
    consts = nc.dram_tensor("consts", [128, 1], f32, kind="ExternalInput").ap()
    brini = nc.dram_tensor("brini", [8, PIXH], f32r, kind="ExternalInput").ap()
    outd = nc.dram_tensor("out", [SL, 2, PIXH], f32, kind="ExternalOutput").ap()
    # dram scratch for layout shuffles
    nsq_scr = nc.dram_tensor("nsq_scr", [SL, KTOT], f32, kind="Internal").ap()
    pmt_scr = nc.dram_tensor("pmt_scr", [SL, KTOT], f32r, kind="Internal").ap()
    th_scr = nc.dram_tensor("th_scr", [SL, 3, PIX], f32r, kind="Internal").ap()
    if DEBUG:
        dbg_pmt = nc.dram_tensor("dbg_pmt", [SL, 8, L], f32r, kind="ExternalOutput").ap()
        dbg_ms = nc.dram_tensor("dbg_ms", [SL, 2, 96, 96], f32r, kind="ExternalOutput").ap()
        dbg_nsq = nc.dram_tensor("dbg_nsq", [SL, 128, NKT], f32, kind="ExternalOutput").ap()

    with tile.TileContext(nc) as tc, ExitStack() as ctx:
        big = ctx.enter_context(tc.tile_pool(name="big", bufs=2))
        sqp = ctx.enter_context(tc.tile_pool(name="sqp", bufs=2))
        sml = ctx.enter_context(tc.tile_pool(name="sml", bufs=2))
        nrow = ctx.enter_context(tc.tile_pool(name="nrow", bufs=2))
        epool = ctx.enter_context(tc.tile_pool(name="epool", bufs=4))
        cst = ctx.enter_context(tc.tile_pool(name="cst", bufs=1))
        tails = ctx.enter_context(tc.tile_pool(name="tails", bufs=2))
        pmtp = ctx.enter_context(tc.tile_pool(name="pmtp", bufs=1))
        lmp = ctx.enter_context(tc.tile_pool(name="lmp", bufs=1))
        chks = ctx.enter_context(tc.tile_pool(name="chks", bufs=2))
        simp = ctx.enter_context(tc.tile_pool(name="simp", bufs=2, space="PSUM"))
        ndp = ctx.enter_context(tc.tile_pool(name="ndp", bufs=1, space="PSUM"))
        misc = ctx.enter_context(tc.tile_pool(name="misc", bufs=2, space="PSUM"))

        def mtile(shape):
            t = misc.tile(shape, f32, tag="misc")
            return t

        # persistent constants
        w1sb = cst.tile([128, 128], bf16); nc.sync.dma_start(w1sb[:], w1[:, :])
        w2sb = cst.tile([128, 2], bf16);   nc.sync.dma_start(w2sb[:], w2[:, :])
        b1sb = cst.tile([128, 1], f32);    nc.sync.dma_start(b1sb[:], b1[:, :])
        blsb = cst.tile([8, 128], f32r);   nc.sync.dma_start(blsb[:], binlhs[:, :])
        utsb = cst.tile([24, 96], f32r);   nc.sync.dma_start(utsb[:], utm[:, :])
        onsb = cst.tile([128, 1], f32r);   nc.sync.dma_start(onsb[:], ones128[:, :])
        orow = cst.tile([1, 128], f32r);   nc.sync.dma_start(orow[:], onesrow[:, :])
        cstsb = cst.tile([128, 1], f32);   nc.sync.dma_start(cstsb[:], consts[:, :])
        zsb = cst.tile([128, 1], f32);     nc.vector.memset(zsb[:], 0.0)
        brhs = cst.tile([8, PIXH], f32r)
        nc.sync.dma_start(brhs[:], brini[:, :])

        S = [dict() for _ in range(SL)]  # per-seq tile handles (reset each rep)

        def load(s):
            d = S[s]
            te0_t = sml.tile([128, L], f32, tag="te0")
            nc.sync.dma_start(te0_t[:], tef[s, 0])
            te1_t = sml.tile([128, L], f32, tag="te1")
            nc.sync.dma_start(te1_t[:], tef[s, 1])
            d["te0"], d["te1"] = te0_t, te1_t
            tr0_t = big.tile([128, KTOT], f32r, tag="tr0")
            tr1_t = big.tile([128, KTOT], f32r, tag="tr1")
            d["tr0"], d["tr1"] = tr0_t, tr1_t
            for q0 in range(0, KTOT, 1152):
                nc.sync.dma_start(tr0_t[:, q0:q0 + 1152], trf[s, 0, :, q0:q0 + 1152])
                nc.sync.dma_start(tr1_t[:, q0:q0 + 1152], trf[s, 1, :, q0:q0 + 1152])

        def load2(s):
            d = S[s]
            lmsb_t = lmp.tile([128, NKT * 40], f32r, tag="lmsb")
            nc.sync.dma_start(lmsb_t[:], lmat[s])
            scsb_t = sml.tile([96, 96], f32, tag="scsb")
            nc.sync.dma_start(scsb_t[:], tsc[s])
            d["lmsb"], d["scsb"] = lmsb_t, scsb_t

        def techain(s):
            d = S[s]
            te0, te1 = d["te0"], d["te1"]
            tq0 = sqp.tile([128, 576], f32r, tag="sqc0")
            nc.vector.tensor_tensor(tq0[:], te0[:], te0[:], op=ALU.mult)
            tq1 = sqp.tile([128, 576], f32r, tag="sqc1")
            nc.vector.tensor_tensor(tq1[:], te1[:], te1[:], op=ALU.mult)
            nsq_q = sml.tile([1, L], f32, tag="nsq_q")
            for o, w in ((0, 512), (512, 64)):
                pnT = mtile([1, 512])
                nc.tensor.matmul(pnT[0:1, 0:w], onsb[:, 0:1], tq0[:, o:o + w],
                                 start=True, stop=False)
                nc.tensor.matmul(pnT[0:1, 0:w], onsb[:, 0:1], tq1[:, o:o + w],
                                 start=False, stop=True)
                nc.vector.tensor_copy(nsq_q[0:1, o:o + w], pnT[0:1, 0:w])
            rq = _rsqrt(nc, sml, nsq_q[:], 1, L, tag="q")
            rq_r = sml.tile([1, L], f32r, tag="rq_r")
            nc.vector.tensor_copy(rq_r[:], rq[:])
            tes0_t = sml.tile([128, L], f32r, tag="tes0")
            tes1_t = sml.tile([128, L], f32r, tag="tes1")
            d["tes0"], d["tes1"] = tes0_t, tes1_t
            for o, w in ((0, 512), (512, 64)):
                bcp = mtile([128, 512])
                nc.tensor.matmul(bcp[:, 0:w], orow[:], rq_r[0:1, o:o + w],
                                 start=True, stop=True)
                nc.vector.tensor_tensor(tes0_t[:, o:o + w], te0[:, o:o + w],
                                        bcp[:, 0:w], op=ALU.mult)
                nc.vector.tensor_tensor(tes1_t[:, o:o + w], te1[:, o:o + w],
                                        bcp[:, 0:w], op=ALU.mult)

        def norms(s):
            d = S[s]
            tr0, tr1 = d["tr0"], d["tr1"]
            sclk = sml.tile([128, NKT], f32, tag="sclk")
            nsqk = sml.tile([128, NKT], f32, tag="nsqk")
            CH = 1024 if s == 0 else 512
            for bi, (k0, k1) in enumerate(((0, 512), (512, 1536), (1536, 3072), (3072, KTOT))):
                for r0 in range(k0, k1, CH):
                    rw = min(CH, k1 - r0)
                    if s == 0:
                        pn = simp.tile([1, 1024], f32, tag="sps")
                    else:
                        pn = mtile([1, 512])
                    for j in range(0, rw, 512):
                        w = min(512, rw - j)
                        sqc0 = sqp.tile([128, 512], f32r, tag="sqc0")
                        sqc1 = sqp.tile([128, 512], f32r, tag="sqc1")
                        if s == 0:
                            nc.scalar.activation(sqc0[:, 0:w], tr0[:, r0 + j:r0 + j + w],
                                                 AF.Square, bias=zsb[:, 0:1])
                        else:
                            nc.vector.tensor_tensor(sqc0[:, 0:w], tr0[:, r0 + j:r0 + j + w],
                                                    tr0[:, r0 + j:r0 + j + w], op=ALU.mult)
                        nc.vector.tensor_tensor(sqc1[:, 0:w], tr1[:, r0 + j:r0 + j + w],
                                                tr1[:, r0 + j:r0 + j + w], op=ALU.mult)
                        nc.tensor.matmul(pn[0:1, j:j + w], onsb[:, 0:1], sqc0[:, 0:w],
                                         start=True, stop=False)
                        nc.tensor.matmul(pn[0:1, j:j + w], onsb[:, 0:1], sqc1[:, 0:w],
                                         start=False, stop=True)
                    nsq_row = nrow.tile([1, 1024], f32, tag="nsq_row")
                    if s == 0:
                        nc.scalar.copy(nsq_row[0:1, 0:rw], pn[0:1, 0:rw])
                    else:
                        nc.vector.tensor_copy(nsq_row[0:1, 0:rw], pn[0:1, 0:rw])
                    nc.sync.dma_start(nsq_scr[s, r0:r0 + rw], nsq_row[0:1, 0:rw])
                t0_, t1_ = k0 // 128, k1 // 128
                nc.sync.dma_start(
                    nsqk[:, t0_:t1_],
                    nsq_scr[s, k0:k1].rearrange("(t p) -> p t", p=128))
                rk = _rsqrt(nc, sml, nsqk[:, t0_:t1_], 128, t1_ - t0_, tag=f"k{bi}")
                nc.vector.tensor_scalar(sclk[:, t0_:t1_], rk[:], cstsb[:, 0:1],
                                        None, op0=ALU.mult)
            if DEBUG:
                nc.sync.dma_start(dbg_nsq[s], nsqk[:])
            d["sclk"] = sclk

        def sim(s, weave=None):
            d = S[s]
            tr0, tr1, tes0, tes1 = d["tr0"], d["tr1"], d["tes0"], d["tes1"]
            lmsb, sclk = d["lmsb"], d["sclk"]
            ndps = ndp.tile([40, 2, 512], f32, tag="ndps")
            d["ndps"] = ndps

            def nd_mm(t, et):
                for n in range(2):
                    nc.tensor.matmul(ndps[:, n, 0:288],
                                     lmsb[:, t * 40:(t + 1) * 40],
                                     et[:, n, 0:288],
                                     start=(t == 0), stop=(t == NKT - 1))

            prev = None
            for t in range(NKT):
                sps = simp.tile([128, 2, 512], f32, tag="sps")
                for n, no in enumerate((0, 288)):
                    nc.tensor.matmul(sps[:, n, 0:288], tr0[:, t * 128:(t + 1) * 128],
                                     tes0[:, no:no + 288], start=True, stop=False)
                    nc.tensor.matmul(sps[:, n, 0:288], tr1[:, t * 128:(t + 1) * 128],
                                     tes1[:, no:no + 288], start=False, stop=True)
                et = epool.tile([128, 2, 288], f32r, tag="et")
                nc.scalar.activation(et[:], sps[:, 0:2, 0:288], AF.Exp,
                                     bias=zsb[:, 0:1], scale=sclk[:, t:t + 1])
                if prev is not None:
                    nd_mm(t - 1, prev)
                prev = et
                if weave is not None:
                    weave(t)
            nd_mm(NKT - 1, prev)

        def pmtevac(s):
            d = S[s]
            ndps = d["ndps"]
            recd = pmtp.tile([8, 2, 288], f32, tag="recd")
            nc.vector.reciprocal(recd[:], ndps[32:40, 0:2, 0:288])
            pmt = pmtp.tile([8, 2, 288], f32r, tag="pmt")
            nc.vector.tensor_tensor(pmt[:], ndps[0:8, 0:2, 0:288], recd[:], op=ALU.mult)
            if DEBUG:
                nc.sync.dma_start(dbg_pmt[s], pmt[:].rearrange("m n w -> m (n w)"))
            nc.sync.dma_start(pmt_scr[s].rearrange("(m q) -> m q", m=8),
                              pmt[:].rearrange("m n w -> m (n w)"))

        def tailA(s):
            d = S[s]
            scsb = d["scsb"]
            p24 = tails.tile([24, 192], f32r, tag="p24")
            nc.sync.dma_start(
                p24[:].rearrange("i (m j) -> i m j", m=8),
                pmt_scr[s].rearrange("(m i j) -> i m j", m=8, i=24))
            t1t = tails.tile([24, 768], f32r, tag="t1t")
            for half in range(2):
                t1p = mtile([24, 512])
                for mm in range(4):
                    m = half * 4 + mm
                    nc.tensor.matmul(t1p[:, mm * 96:(mm + 1) * 96],
                                     p24[:, m * 24:(m + 1) * 24], utsb[:],
                                     start=True, stop=True)
                nc.vector.tensor_copy(t1t[:, half * 384:(half + 1) * 384],
                                      t1p[:, 0:384])
            uS = tails.tile([96, 768], f32, tag="uS")
            uQ = tails.tile([96, 768], f32, tag="uQ")
            for o, w in ((0, 512), (512, 256)):
                ups = mtile([96, 512])
                nc.tensor.matmul(ups[:, 0:w], utsb[:], t1t[:, o:o + w],
                                 start=True, stop=True)
                nc.vector.tensor_copy(uS[:, o:o + w], ups[:, 0:w])
                nc.scalar.activation(uQ[:, o:o + w], ups[:, 0:w], AF.Square,
                                     bias=zsb[0:96, 0:1])
            sU = tails.tile([96, 96], f32, tag="sU")
            nc.vector.tensor_reduce(sU[:], uS[:].rearrange("x (m y) -> x y m", m=8),
                                    axis=mybir.AxisListType.X, op=ALU.add)
            sQ = tails.tile([96, 96], f32, tag="sQ")
            nc.vector.tensor_reduce(sQ[:], uQ[:].rearrange("x (m y) -> x y m", m=8),
                                    axis=mybir.AxisListType.X, op=ALU.add)
            m2 = tails.tile([96, 96], f32, tag="m2")
            nc.vector.tensor_tensor(m2[:], sU[:], sU[:], op=ALU.mult)
            nc.vector.tensor_scalar(m2[:], m2[:], -0.125, None, op0=ALU.mult)
            nc.vector.tensor_tensor(m2[:], m2[:], sQ[:], op=ALU.add)
            nc.vector.tensor_scalar(m2[:], m2[:], 1.0 / 7.0, 1e-30,
                                    op0=ALU.mult, op1=ALU.max)
            rv = _rsqrt(nc, tails, m2[:], 96, 96, tag="v")
            stdv = tails.tile([96, 96], f32, tag="stdv")
            nc.vector.tensor_tensor(stdv[:], m2[:], rv[:], op=ALU.mult)
            th_s = tails.tile([96, 96], f32r, tag="th_s")
            nc.scalar.activation(th_s[:], scsb[:], AF.Tanh, bias=zsb[0:96, 0:1])
            th_m = tails.tile([96, 96], f32r, tag="th_m")
            nc.scalar.activation(th_m[:], sU[:], AF.Tanh, bias=zsb[0:96, 0:1],
                                 scale=0.125)
            th_d = tails.tile([96, 96], f32r, tag="th_d")
            nc.scalar.activation(th_d[:], stdv[:], AF.Tanh, bias=zsb[0:96, 0:1])
            if DEBUG:
                nc.sync.dma_start(dbg_ms[s, 0], th_m[:])
                nc.sync.dma_start(dbg_ms[s, 1], th_d[:])
            d["th"] = (th_s, th_m, th_d)

        def tailF(s):
            d = S[s]
            for j, th in enumerate(d["th"]):
                nc.sync.dma_start(th_scr[s, j].rearrange("(a b) -> a b", a=96), th[:])
                for g in range(2):
                    nc.sync.dma_start(brhs[4 * g + j:4 * g + j + 1, :],
                                      th_scr[s, j, g * PIXH:(g + 1) * PIXH])

        def tailB_chunk(s, pc):
            d = S[s]
            c2sb = d["c2sb"]
            cw = min(1024, PIXH - pc)
            nmm = (cw + 511) // 512
            dpps = simp.tile([128, 2, 512], f32, tag="sps")
            for n in range(nmm):
                w = min(512, cw - n * 512)
                nc.tensor.matmul(dpps[:, n, 0:w], blsb[:],
                                 brhs[:, pc + n * 512:pc + n * 512 + w],
                                 start=True, stop=True)
            e1 = chks.tile([128, 1024], bf16, tag="e1")
            nc.scalar.activation(
                e1[:, 0:cw].rearrange("p (n w) -> p n w", n=nmm) if nmm > 1 else e1[:, 0:cw],
                dpps[:, 0:nmm, 0:512] if nmm > 1 else dpps[:, 0, 0:cw],
                AF.Abs, bias=zsb[:, 0:1])
            enc = chks.tile([128, 1024], bf16, tag="enc")
            nc.vector.tensor_scalar(enc[:, 0:cw], e1[:, 0:cw], -1.0, -1.0,
                                    op0=ALU.mult, op1=ALU.max)
            c1ps = simp.tile([128, 2, 512], f32, tag="sps")
            for n in range(nmm):
                w = min(512, cw - n * 512)
                nc.tensor.matmul(c1ps[:, n, 0:w], w1sb[:],
                                 enc[:, n * 512:n * 512 + w], start=True, stop=True)
            r1 = chks.tile([128, 1024], bf16, tag="r1")
            nc.scalar.activation(
                r1[:, 0:cw].rearrange("p (n w) -> p n w", n=nmm) if nmm > 1 else r1[:, 0:cw],
                c1ps[:, 0:nmm, 0:512] if nmm > 1 else c1ps[:, 0, 0:cw],
                AF.Relu, bias=b1sb[:, 0:1])
            c2ps = mtile([128, 16])
            for n in range(0, cw, 128):
                nc.tensor.matmul(c2ps[:, 2 * (n // 128):2 * (n // 128) + 2],
                                 r1[:, n:n + 128], w2sb[:], start=True, stop=True)
            ci0 = pc // 128
            ncn = cw // 128
            nc.vector.tensor_copy(
                c2sb[:, :].rearrange("p (g c) -> p c g", g=2)[:, ci0:ci0 + ncn, :],
                c2ps[:, 0:2 * ncn].rearrange("p (c g) -> p c g", g=2))

        def tailB_start(s):
            d = S[s]
            c2sb_t = tails.tile([128, 72], f32, tag="c2sb")
            d["c2sb"] = c2sb_t

        def tailB_finish(s):
            d = S[s]
            nc.sync.dma_start(
                outd[s].rearrange("g (c p) -> p g c", p=128),
                d["c2sb"][:].rearrange("p (g c) -> p g c", g=2))

        # interleaved schedule for cross-sequence overlap
        for _rep in range(REPEAT):
            for d in S:
                d.clear()
            load(0); techain(0); norms(0); load2(0); sim(0)
            load(1); techain(1); norms(1); load2(1)
            pmtevac(0)
            tailA(0)
            tailF(0)
            sim(1)
            pmtevac(1)
            tailA(1)
            tailB_start(0)
            for pc in range(0, PIXH, 1024):
                tailB_chunk(0, pc)
            tailB_finish(0)
            tailF(1)
            tailB_start(1)
            for pc in range(0, PIXH, 1024):
                tailB_chunk(1, pc)
            tailB_finish(1)

    nc.compile()
    return nc


_prog = None


def kernel(**inputs) -> np.ndarray:
    global _prog
    test_scores = np.asarray(inputs["test_scores"], np.float32)
    train_labels = np.asarray(inputs["train_labels"], np.float32)
    test_feat = np.asarray(inputs["test_feat"], np.float32)
    train_feats = np.asarray(inputs["train_feats"], np.float32)
    temp = float(np.asarray(inputs["softmax_temp"]).reshape(-1)[0])
    conv1_w = np.asarray(inputs["conv1_w"], np.float32)
    conv1_b = np.asarray(inputs["conv1_b"], np.float32)
    bn_gamma = np.asarray(inputs["bn_gamma"], np.float32)
    bn_beta = np.asarray(inputs["bn_beta"], np.float32)
    bn_mean = np.asarray(inputs["bn_mean"], np.float32)
    bn_var = np.asarray(inputs["bn_var"], np.float32)
    conv2_w = np.asarray(inputs["conv2_w"], np.float32)
    conv2_b = np.asarray(inputs["conv2_b"], np.float32)

    import ml_dtypes

    R = resize_matrix(96, 24)
    U = resize_matrix(24, 96)
    labd = np.einsum("ik,mskl,jl->msij", R, train_labels, R)  # (M, NSEQ, 24, 24)
    lm_all = np.zeros((NSEQ, KTOT, 40), np.float32)
    for m in range(M):
        lm_all[:, m * L:(m + 1) * L, m] = labd[m].reshape(NSEQ, L)
        lm_all[:, m * L:(m + 1) * L, 32 + m] = 1.0
    lm_dev = lm_all.reshape(NSEQ, NKT, 128, 40).transpose(0, 2, 1, 3) \
        .reshape(NSEQ, 128, NKT * 40)

    s_o = np.sqrt(bn_var + BN_EPS)
    w1f = conv1_w * (bn_gamma / s_o)[:, None]
    b1f = (conv1_b - bn_mean) / s_o * bn_gamma + bn_beta
    b1f = b1f + w1f.sum(axis=1)   # kernel feeds enc-1; fold +1*W1 into bias
    W1 = np.zeros((128, 128), np.float32)
    W1[0:64, 0:64] = w1f.T
    W1[64:128, 64:128] = w1f.T
    W2 = np.zeros((128, 2), np.float32)
    W2[0:64, 0] = conv2_w[0]
    W2[64:128, 1] = conv2_w[0]
    B1 = np.concatenate([b1f, b1f]).reshape(128, 1)

    BL = np.zeros((8, 128), np.float32)
    for g in range(2):
        for ch in range(64):
            p = ch + 64 * g
            if ch < 32:
                j, a, b, c = 0, 15.5, 15.5, float(ch)
            elif ch < 48:
                j, a, b, c = 1, 15.0, 0.0, float(ch - 32)
            else:
                j, a, b, c = 2, 15.0, 0.0, float(ch - 48)
            BL[4 * g + j, p] = a
            BL[4 * g + 3, p] += b - c

    UT = np.ascontiguousarray(U.T)
    CONSTS = np.full((128, 1), temp, np.float32)

    tf_r = train_feats.reshape(M, NSEQ, C, L)
    te_r = test_feat.reshape(NSEQ, C, L)
    in_maps = []
    for c in range(NCORES):
        sl = slice(SL * c, SL * (c + 1))
        trc = np.ascontiguousarray(
            tf_r[:, sl].transpose(1, 2, 0, 3).reshape(SL, 2, 128, KTOT))
        tec = np.ascontiguousarray(te_r[sl].reshape(SL, 2, 128, L))
        tscc = np.ascontiguousarray(np.transpose(test_scores[0, sl], (0, 2, 1)))
        in_maps.append({
            "trf": trc, "tef": tec,
            "lmat": np.ascontiguousarray(lm_dev[sl]),
            "tsc": tscc,
            "w1": W1.astype(ml_dtypes.bfloat16),
            "w2": W2.astype(ml_dtypes.bfloat16),
            "b1": B1, "binlhs": BL, "utm": UT,
            "ones128": np.ones((128, 1), np.float32),
            "onesrow": np.ones((1, 128), np.float32),
            "consts": CONSTS,
            "brini": np.ones((8, PIXH), np.float32),
        })

    if _prog is None:
        _prog = build_program()
    res = run_bass_kernel_spmd(_prog, in_maps, core_ids=list(range(NCORES)))

    out = np.empty((1, NSEQ, WL, HL), np.float32)
    for c in range(NCORES):
        o = res.results[c]["out"]
        for s in range(SL):
            img_t = np.concatenate([o[s, 0], o[s, 1]]).reshape(96, 96)
            out[0, SL * c + s] = img_t.T + conv2_b[0]
    if DEBUG:
        kernel._last_debug = [res.results[c] for c in range(NCORES)]
    return out


if __name__ == "__main__":
    rng = np.random.default_rng(0)
    ins = {
        "test_scores": rng.standard_normal((1, NSEQ, WL, HL)).astype(np.float32),
        "train_labels": rng.uniform(0, 1, (M, NSEQ, WL, HL)).astype(np.float32),
        "test_feat": rng.standard_normal((1, NSEQ, C, WF, HF)).astype(np.float32),
        "train_feats": rng.standard_normal((M, NSEQ, C, WF, HF)).astype(np.float32),
        "softmax_temp": np.full((1,), 50.0, np.float32),
        "conv1_w": (rng.standard_normal((64, 64)) * 0.05).astype(np.float32),
        "conv1_b": np.zeros((64,), np.float32),
        "bn_gamma": np.ones((64,), np.float32),
        "bn_beta": np.zeros((64,), np.float32),
        "bn_mean": np.zeros((64,), np.float32),
        "bn_var": np.ones((64,), np.float32),
        "conv2_w": (rng.standard_normal((1, 64)) * 0.05).astype(np.float32),
        "conv2_b": np.zeros((1,), np.float32),
    }
    out = kernel(**ins)
    print("out shape:", out.shape, "mean", float(out.mean()), "std", float(out.std()))



# revision 48
# speedup vs baseline: 2.7158x; 2.7158x over previous
"""Trainium2 Bass kernel for nn_AttentionLearnFusionDirectModule (v2).

Takes FULL inputs, returns FULL output. Shards nseq=16 across 8 NeuronCores
(2 seqs/core, pure data parallel), runs one SPMD Bass program via
bass_utils.run_bass_kernel_spmd, gathers the output on host.

v2 vs v1:
  - te/tr cosine norms folded on the HOST (features shipped pre-normalized,
    bf16) -> the whole norms/techain front-end is gone and the exp scale is
    the scalar softmax temp.
  - bf16 features halve the HBM load (9.4MB/core for train feats).
  - convs/enc in f32r instead of bf16 (same PE rate at >=256-wide moving,
    much better accuracy), labels stay f32r.
  - binning ones-rows folded into the Abs activation per-partition bias.
  - staging shuffles bounce through DRAM on the (otherwise idle) Pool/SWDGE
    queue so they never block the SP input-load queue.
  - num/den matmul lags the sim by 4 k-tiles so the label matrix DMA is off
    the critical path.

Per-core pipeline (per sequence):
  1. cosine sim: bf16 matmuls (c,k)x(c,q) per 128-wide k-tile
  2. softmax-free attention: E = exp(temp*cos) on ACT; num/den via one
     40-col block-label matmul accumulated over all k-tiles (lagged)
  3. pmt = num * recip(den); bilinear 24->96 upsample as two PE matmuls
  4. mean/std over memories, tanh, triangular soft-binning via a PE
     bin-center matmul + Abs(+bias), then f32r 1x1 convs (BN folded)
"""
import os
import sys

sys.path.insert(0, '/opt/trn_rl_repo')

import numpy as np
from contextlib import ExitStack

import concourse.bass as bass
import concourse.tile as tile
from concourse import mybir, bacc
from concourse.bass_utils import run_bass_kernel_spmd

f32 = mybir.dt.float32
f32r = mybir.dt.float32r
bf16 = mybir.dt.bfloat16
i32 = mybir.dt.int32
AF = mybir.ActivationFunctionType
ALU = mybir.AluOpType

M = 8
NSEQ = 16
C = 256
WF = HF = 24
WL = HL = 96
L = WF * HF            # 576
KTOT = M * L           # 4608
NCORES = 8
SL = NSEQ // NCORES    # 2
NKT = KTOT // 128      # 36
PIX = WL * HL          # 9216
PIXH = PIX // 2        # 4608
MAGIC = 0x5F3759DF
TEMP_DEFAULT = 50.0
LAG = 6                # nd matmul lags sim by this many k-tiles
REPEAT = int(os.environ.get("KERNEL_REPEAT", "1"))


def resize_matrix(n_in, n_out):
    """Row-normalized triangle-kernel resize matrix matching jax.image.resize
    bilinear (antialias on downscale, half-pixel centers)."""
    scale = n_out / n_in
    kscale = min(scale, 1.0)
    R = np.zeros((n_out, n_in), np.float64)
    for i in range(n_out):
        c = (i + 0.5) / scale - 0.5
        w = np.maximum(0.0, 1.0 - np.abs(np.arange(n_in) - c) * kscale)
        R[i] = w / w.sum()
    return R.astype(np.float32)


def _rsqrt(nc, pool, x_ap, P, F, tag, n_iter=2):
    """1/sqrt(x) on DVE only (quake init + Newton). x: fp32 SBUF AP."""
    y = pool.tile([P, F], f32, tag=f"rsq_y{tag}")
    t1 = pool.tile([P, F], f32, tag=f"rsq_t{tag}")
    yi = y[:].bitcast(i32)
    nc.vector.tensor_scalar(yi, x_ap.bitcast(i32), 1, None,
                            op0=ALU.logical_shift_right)
    nc.vector.tensor_scalar(yi, yi, -1, None, op0=ALU.bitwise_xor)
    nc.vector.tensor_scalar(yi, yi, MAGIC + 1, None, op0=ALU.add)
    for _ in range(n_iter):
        nc.vector.tensor_tensor(t1[:], y[:], y[:], op=ALU.mult)
        nc.vector.tensor_tensor(t1[:], t1[:], x_ap, op=ALU.mult)
        nc.vector.tensor_scalar(t1[:], t1[:], -0.5, 1.5, op0=ALU.mult, op1=ALU.add)
        nc.vector.tensor_tensor(y[:], y[:], t1[:], op=ALU.mult)
    return y


def build_program(temp=TEMP_DEFAULT):
    nc = bacc.Bacc("TRN2", target_bir_lowering=False, debug=False,
                   num_devices=NCORES)

    trf = nc.dram_tensor("trf", [SL, 2, 128, KTOT], bf16, kind="ExternalInput").ap()
    tef = nc.dram_tensor("tef", [SL, 2, 128, L], bf16, kind="ExternalInput").ap()
    lmat = nc.dram_tensor("lmat", [SL, 128, NKT * 40], f32r, kind="ExternalInput").ap()
    tsc = nc.dram_tensor("tsc", [SL, 96, 96], f32, kind="ExternalInput").ap()
    w1 = nc.dram_tensor("w1", [128, 128], f32r, kind="ExternalInput").ap()
    w2 = nc.dram_tensor("w2", [128, 2], f32r, kind="ExternalInput").ap()
    b1 = nc.dram_tensor("b1", [128, 1], f32, kind="ExternalInput").ap()
    bc = nc.dram_tensor("bc", [128, 1], f32, kind="ExternalInput").ap()
    bl6 = nc.dram_tensor("bl6", [128, 128], f32r, kind="ExternalInput").ap()
    utm = nc.dram_tensor("utm", [24, 96], f32r, kind="ExternalInput").ap()
    outd = nc.dram_tensor("out", [SL, 128, 72], f32, kind="ExternalOutput").ap()
    # DRAM scratch for partition-reshuffle staging (Pool queue); one tensor
    # per sequence so the race detector sees disjoint footprints
    pmt_scr = [nc.dram_tensor(f"pmt_scr{s}", [KTOT], f32r, kind="Internal").ap()
               for s in range(SL)]
    th_scr = [nc.dram_tensor(f"th_scr{s}", [3, PIX], f32r, kind="Internal").ap()
              for s in range(SL)]

    with tile.TileContext(nc) as tc, ExitStack() as ctx:
        big = ctx.enter_context(tc.tile_pool(name="big", bufs=2))
        sml = ctx.enter_context(tc.tile_pool(name="sml", bufs=2))
        epool = ctx.enter_context(tc.tile_pool(name="epool", bufs=2 * (LAG + 2)))
        cst = ctx.enter_context(tc.tile_pool(name="cst", bufs=1))
        tails0 = ctx.enter_context(tc.tile_pool(name="tails0", bufs=1))
        tails1 = ctx.enter_context(tc.tile_pool(name="tails1", bufs=1))
        tpool = (tails0, tails1)
        # dead 4KB spacer isolates the tails0/tails1 boundary from shadow
        # granule false-sharing (late brhs DMA write vs seq1 tail reads)
        pad0 = tails1.tile([128, 1024], f32, tag="pad")
        lmp = ctx.enter_context(tc.tile_pool(name="lmp", bufs=2))
        chks = ctx.enter_context(tc.tile_pool(name="chks", bufs=2))
        simp = ctx.enter_context(tc.tile_pool(name="simp", bufs=2, space="PSUM"))
        ndp = ctx.enter_context(tc.tile_pool(name="ndp", bufs=1, space="PSUM"))
        misc = ctx.enter_context(tc.tile_pool(name="misc", bufs=1, space="PSUM"))

        # persistent constants; tiles created up front, DMAs issued by
        # load_consts() after the first critical input chunks (only zsb is
        # needed at sim start and it's a DVE memset)
        w1sb = cst.tile([128, 128], f32r)
        w2sb = cst.tile([128, 2], f32r)
        b1sb = cst.tile([128, 1], f32)
        bcsb = cst.tile([128, 1], f32)
        blsb = cst.tile([128, 128], f32r)
        utsb = cst.tile([24, 96], f32r)
        zsb = cst.tile([128, 1], f32);     nc.vector.memset(zsb[:], 0.0)

        def load_consts():
            nc.sync.dma_start(w1sb[:], w1[:, :])
            nc.sync.dma_start(w2sb[:], w2[:, :])
            nc.sync.dma_start(b1sb[:], b1[:, :])
            nc.sync.dma_start(bcsb[:], bc[:, :])
            nc.sync.dma_start(blsb[:], bl6[:, :])
            nc.sync.dma_start(utsb[:], utm[:, :])

        S = [dict() for _ in range(SL)]  # per-seq tile handles (reset each rep)

        def load(s):
            """Criticality-ordered input DMAs for one sequence (SP queue)."""
            d = S[s]
            te_t = sml.tile([128, 2, L], bf16, tag="te")
            nc.sync.dma_start(te_t[:], tef[s].rearrange("c p q -> p c q"))
            d["te"] = te_t
            tr0_t = big.tile([128, KTOT], bf16, tag="tr0")
            tr1_t = big.tile([128, KTOT], bf16, tag="tr1")
            d["tr0"], d["tr1"] = tr0_t, tr1_t
            lmsb_t = lmp.tile([128, NKT * 40], f32r, tag="lmsb")
            d["lmsb"] = lmsb_t
            scsb_t = sml.tile([96, 96], f32, tag="scsb")
            d["scsb"] = scsb_t

            def chunk(q0, q1):
                nc.sync.dma_start(tr0_t[:, q0:q1], trf[s, 0, :, q0:q1])
                nc.sync.dma_start(tr1_t[:, q0:q1], trf[s, 1, :, q0:q1])

            chunk(0, 576)
            chunk(576, 1920)
            if s == 0:
                load_consts()
            nc.sync.dma_start(lmsb_t[:, 0:18 * 40], lmat[s, :, 0:18 * 40])
            chunk(1920, 3264)
            nc.sync.dma_start(lmsb_t[:, 18 * 40:], lmat[s, :, 18 * 40:])
            chunk(3264, 4608)
            nc.sync.dma_start(scsb_t[:], tsc[s])

        def sim(s, weave=None):
            d = S[s]
            tr0, tr1, te, lmsb = d["tr0"], d["tr1"], d["te"], d["lmsb"]
            ndps = ndp.tile([40, 2, 512], f32, tag="ndps")
            d["ndps"] = ndps
            ets = {}

            def nd_mm(t):
                for n in range(2):
                    nc.tensor.matmul(ndps[:, n, 0:288],
                                     lmsb[:, t * 40:(t + 1) * 40],
                                     ets[t][n],
                                     start=(t == 0), stop=(t == NKT - 1))
                del ets[t]

            for t in range(NKT):
                sps = simp.tile([128, 2, 512], f32, tag="sps")
                for n, no in enumerate((0, 288)):
                    nc.tensor.matmul(sps[:, n, 0:288], tr0[:, t * 128:(t + 1) * 128],
                                     te[:, 0, no:no + 288], start=True, stop=False)
                    nc.tensor.matmul(sps[:, n, 0:288], tr1[:, t * 128:(t + 1) * 128],
                                     te[:, 1, no:no + 288], start=False, stop=True)
                et2 = epool.tile([128, 2, 288], f32r, tag="et")
                nc.scalar.activation(et2[:], sps[:, 0:2, 0:288], AF.Exp,
                                     bias=zsb[:, 0:1], scale=float(temp))
                ets[t] = (et2[:, 0], et2[:, 1])
                if t >= LAG:
                    nd_mm(t - LAG)
                if weave is not None:
                    weave(t)
            for t in range(NKT - LAG, NKT):
                nd_mm(t)

        def pmtevac(s, q):
            d = S[s]
            tails = tpool[s]
            ndps = d["ndps"]
            recd = tails.tile([8, 2, 288], f32, tag="recd")
            nc.vector.reciprocal(recd[:], ndps[32:40, 0:2, 0:288])
            pmt = tails.tile([8, 2, 288], f32r, tag="pmt")
            nc.vector.tensor_tensor(pmt[:], ndps[0:8, 0:2, 0:288], recd[:],
                                    op=ALU.mult)
            q.dma_start(pmt_scr[s][:].rearrange("(m q) -> m q", m=8),
                        pmt[:].rearrange("m n w -> m (n w)"))

        def stage24(s, q):
            d = S[s]
            tails = tpool[s]
            p24 = tails.tile([24, 192], f32r, tag="p24")
            q.dma_start(
                p24[:].rearrange("i (m j) -> i m j", m=8),
                pmt_scr[s][:].rearrange("(m i j) -> i m j", m=8, i=24))
            d["p24"] = p24

        def tailA(s, q):
            d = S[s]
            tails = tpool[s]
            scsb, p24 = d["scsb"], d["p24"]
            t1t = tails.tile([24, 768], f32r, tag="t1t")
            for half in range(2):
                t1p = misc.tile([128, 512], f32, tag="mA")
                for mm in range(4):
                    m = half * 4 + mm
                    nc.tensor.matmul(t1p[0:24, mm * 96:(mm + 1) * 96],
                                     p24[:, m * 24:(m + 1) * 24], utsb[:],
                                     start=True, stop=True)
                nc.vector.tensor_copy(t1t[:, half * 384:(half + 1) * 384],
                                      t1p[0:24, 0:384])
            uS = tails.tile([96, 768], f32, tag="uS")
            uQ = tails.tile([96, 768], f32, tag="uQ")
            for o, w in ((0, 512), (512, 256)):
                ups = misc.tile([128, 512], f32, tag="mB")
                nc.tensor.matmul(ups[0:96, 0:w], utsb[:], t1t[:, o:o + w],
                                 start=True, stop=True)
                nc.vector.tensor_copy(uS[:, o:o + w], ups[0:96, 0:w])
                nc.vector.tensor_tensor(uQ[:, o:o + w], uS[:, o:o + w],
                                        uS[:, o:o + w], op=ALU.mult)
            sU = tails.tile([96, 96], f32, tag="sU")
            nc.vector.tensor_reduce(sU[:], uS[:].rearrange("x (m y) -> x y m", m=8),
                                    axis=mybir.AxisListType.X, op=ALU.add)
            sQ = tails.tile([96, 96], f32, tag="sQ")
            nc.vector.tensor_reduce(sQ[:], uQ[:].rearrange("x (m y) -> x y m", m=8),
                                    axis=mybir.AxisListType.X, op=ALU.add)
            m2 = tails.tile([96, 96], f32, tag="m2")
            nc.vector.tensor_tensor(m2[:], sU[:], sU[:], op=ALU.mult)
            nc.vector.tensor_scalar(m2[:], m2[:], -0.125, None, op0=ALU.mult)
            nc.vector.tensor_tensor(m2[:], m2[:], sQ[:], op=ALU.add)
            nc.vector.tensor_scalar(m2[:], m2[:], 1.0 / 7.0, 1e-30,
                                    op0=ALU.mult, op1=ALU.max)
            rv = _rsqrt(nc, tails, m2[:], 96, 96, tag="v")
            stdv = tails.tile([96, 96], f32, tag="stdv")
            nc.vector.tensor_tensor(stdv[:], m2[:], rv[:], op=ALU.mult)
            th3 = tails.tile([96, 3, 96], f32r, tag="th3")
            nc.scalar.activation(th3[:, 0, :], scsb[:], AF.Tanh, bias=zsb[0:96, 0:1])
            nc.scalar.activation(th3[:, 1, :], sU[:], AF.Tanh, bias=zsb[0:96, 0:1],
                                 scale=0.125)
            nc.scalar.activation(th3[:, 2, :], stdv[:], AF.Tanh, bias=zsb[0:96, 0:1])
            q.dma_start(th_scr[s][:, :].rearrange("t (a b) -> a t b", a=96), th3[:])
            d["_pin"] = (t1t, uS, uQ, sU, sQ, m2, rv, stdv, th3)

        def tailF(s, q):
            d = S[s]
            tails = tpool[s]
            brhs = tails.tile([6, PIXH], f32r, tag="brhs")
            for g in range(2):
                q.dma_start(brhs[3 * g:3 * g + 3, :],
                            th_scr[s][:, g * PIXH:(g + 1) * PIXH])
            d["brhs"] = brhs

        def tailB_start(s):
            d = S[s]
            c2sb_t = tpool[s].tile([128, 72], f32, tag="c2sb")
            d["c2sb"] = c2sb_t

        def tailB_chunk(s, band, off, cw):
            d = S[s]
            brhs, c2sb = d["brhs"], d["c2sb"]
            pc = band * 1152 + off
            dpps = misc.tile([128, 512], f32, tag="mA")
            nc.tensor.matmul(dpps[:, 0:cw], blsb[0:6, :],
                             brhs[:, pc:pc + cw],
                             start=True, stop=True)
            e1 = chks.tile([128, 512], f32r, tag="e1")
            nc.scalar.activation(e1[:, 0:cw], dpps[:, 0:cw], AF.Abs,
                                 bias=bcsb[:, 0:1])
            enc = chks.tile([128, 512], f32r, tag="enc")
            nc.vector.tensor_scalar(enc[:, 0:cw], e1[:, 0:cw], -1.0, -1.0,
                                    op0=ALU.mult, op1=ALU.max)
            c1ps = misc.tile([128, 512], f32, tag="mB")
            nc.tensor.matmul(c1ps[:, 0:cw], w1sb[:], enc[:, 0:cw],
                             start=True, stop=True)
            r1 = chks.tile([128, 512], f32r, tag="r1")
            nc.vector.tensor_scalar(r1[:, 0:cw], c1ps[:, 0:cw], b1sb[:, 0:1], 0.0,
                                    op0=ALU.add, op1=ALU.max)
            c2ps = misc.tile([128, 512], f32, tag="mB")
            ncn = cw // 128
            for n in range(0, cw, 128):
                nc.tensor.matmul(c2ps[:, 2 * (n // 128):2 * (n // 128) + 2],
                                 r1[:, n:n + 128], w2sb[:], start=True, stop=True)
            ci0 = pc // 128
            nc.vector.tensor_copy(
                c2sb[:, :].rearrange("p (g c) -> p c g", g=2)[:, ci0:ci0 + ncn, :],
                c2ps[:, 0:2 * ncn].rearrange("p (c g) -> p c g", g=2))

        def tailB_finish(s):
            d = S[s]
            nc.sync.dma_start(outd[s], d["c2sb"][:])

        # interleaved schedule for cross-sequence overlap
        for _rep in range(REPEAT):
            for d in S:
                d.clear()
            load(0)
            load(1)
            sim(0)
            pmtevac(0, nc.sync)
            stage24(0, nc.sync)

            # weave seq0's tail into seq1's sim stream
            tail_steps = [lambda: tailA(0, nc.sync), lambda: tailF(0, nc.sync),
                          lambda: tailB_start(0)]
            for g in range(4):
                for off, cw in ((0, 512), (512, 512), (1024, 128)):
                    tail_steps.append(
                        lambda g=g, off=off, cw=cw: tailB_chunk(0, g, off, cw))
            tail_steps.append(lambda: tailB_finish(0))

            step_at = {}
            first, spacing = 5, 2
            for i, fn in enumerate(tail_steps):
                step_at.setdefault(first + i * spacing, []).append(fn)

            def weave(t):
                for fn in step_at.get(t, []):
                    fn()

            sim(1, weave=weave)
            for t in range(NKT, first + len(tail_steps) * spacing + 1):
                for fn in step_at.get(t, []):
                    fn()
            pmtevac(1, nc.sync)
            stage24(1, nc.sync)
            tailA(1, nc.sync)
            tailF(1, nc.sync)
            tailB_start(1)
            for g in range(4):
                for off, cw in ((0, 512), (512, 512), (1024, 128)):
                    tailB_chunk(1, g, off, cw)
            tailB_finish(1)

    nc.compile()
    return nc


_prog = None


def kernel(**inputs) -> np.ndarray:
    global _prog
    test_scores = np.asarray(inputs["test_scores"], np.float32)
    train_labels = np.asarray(inputs["train_labels"], np.float32)
    test_feat = np.asarray(inputs["test_feat"], np.float32)
    train_feats = np.asarray(inputs["train_feats"], np.float32)
    temp = float(np.asarray(inputs["softmax_temp"]).reshape(-1)[0])
    conv1_w = np.asarray(inputs["conv1_w"], np.float32)
    conv1_b = np.asarray(inputs["conv1_b"], np.float32)
    bn_gamma = np.asarray(inputs["bn_gamma"], np.float32)
    bn_beta = np.asarray(inputs["bn_beta"], np.float32)
    bn_mean = np.asarray(inputs["bn_mean"], np.float32)
    bn_var = np.asarray(inputs["bn_var"], np.float32)
    conv2_w = np.asarray(inputs["conv2_w"], np.float32)
    conv2_b = np.asarray(inputs["conv2_b"], np.float32)

    import ml_dtypes

    R = resize_matrix(96, 24)
    labd = np.einsum("ik,mskl,jl->msij", R, train_labels, R)  # (M, NSEQ, 24, 24)
    lm_all = np.zeros((NSEQ, KTOT, 40), np.float32)
    for m in range(M):
        lm_all[:, m * L:(m + 1) * L, m] = labd[m].reshape(NSEQ, L)
        lm_all[:, m * L:(m + 1) * L, 32 + m] = 1.0
    lm_dev = lm_all.reshape(NSEQ, NKT, 128, 40).transpose(0, 2, 1, 3) \
        .reshape(NSEQ, 128, NKT * 40)

    BN_EPS = 1e-5
    s_o = np.sqrt(bn_var + BN_EPS)
    w1f = conv1_w * (bn_gamma / s_o)[:, None]
    b1f = (conv1_b - bn_mean) / s_o * bn_gamma + bn_beta
    b1f = b1f + w1f.sum(axis=1)   # kernel feeds enc-1; fold +1*W1 into bias
    W1 = np.zeros((128, 128), np.float32)
    W1[0:64, 0:64] = w1f.T
    W1[64:128, 64:128] = w1f.T
    W2 = np.zeros((128, 2), np.float32)
    W2[0:64, 0] = conv2_w[0]
    W2[64:128, 1] = conv2_w[0]
    B1 = np.concatenate([b1f, b1f]).reshape(128, 1)

    BL6 = np.zeros((6, 128), np.float32)
    BC = np.zeros((128, 1), np.float32)
    for g in range(2):
        for ch in range(64):
            p = ch + 64 * g
            if ch < 32:
                j, a, b, c = 0, 15.5, 15.5, float(ch)
            elif ch < 48:
                j, a, b, c = 1, 15.0, 0.0, float(ch - 32)
            else:
                j, a, b, c = 2, 15.0, 0.0, float(ch - 48)
            BL6[3 * g + j, p] = a
            BC[p, 0] = b - c
    BL128 = np.zeros((128, 128), np.float32)  # BL6 in rows 0-5 ((half, tensor))
    BL128[0:6, :] = BL6

    UT = np.ascontiguousarray(resize_matrix(24, 96).T)

    # host-side cosine normalization, then bf16
    tf_r = train_feats.reshape(M, NSEQ, C, L)
    tf_n = tf_r / np.sqrt((tf_r * tf_r).sum(axis=2, keepdims=True))
    te_r = test_feat.reshape(NSEQ, C, L)
    te_n = te_r / np.sqrt((te_r * te_r).sum(axis=1, keepdims=True))

    in_maps = []
    for c in range(NCORES):
        sl = slice(SL * c, SL * (c + 1))
        trc = np.ascontiguousarray(
            tf_n[:, sl].transpose(1, 2, 0, 3).reshape(SL, 2, 128, KTOT)
        ).astype(ml_dtypes.bfloat16)
        tec = np.ascontiguousarray(te_n[sl].reshape(SL, 2, 128, L)) \
            .astype(ml_dtypes.bfloat16)
        tscc = np.ascontiguousarray(np.transpose(test_scores[0, sl], (0, 2, 1)))
        in_maps.append({
            "trf": trc, "tef": tec,
            "lmat": np.ascontiguousarray(lm_dev[sl]),
            "tsc": tscc,
            "w1": W1, "w2": W2, "b1": B1, "bc": BC, "bl6": BL128, "utm": UT,
        })

    if _prog is None:
        _prog = build_program(temp)
    res = run_bass_kernel_spmd(_prog, in_maps, core_ids=list(range(NCORES)))

    out = np.empty((1, NSEQ, WL, HL), np.float32)
    for c in range(NCORES):
        o = res.results[c]["out"]   # [SL, 128, (g=2, c=36)] pixel-pair major
        for s in range(SL):
            v = o[s].reshape(128, 2, 36).transpose(1, 2, 0).reshape(2, PIXH)
            img_t = np.concatenate([v[0], v[1]]).reshape(96, 96)
            out[0, SL * c + s] = img_t.T + conv2_b[0]
    return out


if __name__ == "__main__":
    rng = np.random.default_rng(0)
    ins = {
        "test_scores": rng.standard_normal((1, NSEQ, WL, HL)).astype(np.float32),
        "train_labels": rng.uniform(0, 1, (M, NSEQ, WL, HL)).astype(np.float32),
        "test_feat": rng.standard_normal((1, NSEQ, C, WF, HF)).astype(np.float32),
        "train_feats": rng.standard_normal((M, NSEQ, C, WF, HF)).astype(np.float32),
        "softmax_temp": np.full((1,), 50.0, np.float32),
        "conv1_w": (rng.standard_normal((64, 64)) * 0.05).astype(np.float32),
        "conv1_b": np.zeros((64,), np.float32),
        "bn_gamma": np.ones((64,), np.float32),
        "bn_beta": np.zeros((64,), np.float32),
        "bn_mean": np.zeros((64,), np.float32),
        "bn_var": np.ones((64,), np.float32),
        "conv2_w": (rng.standard_normal((1, 64)) * 0.05).astype(np.float32),
        "conv2_b": np.zeros((1,), np.float32),
    }
    out = kernel(**ins)
    print("out shape:", out.shape, "mean", float(out.mean()), "std", float(out.std()))
